# revision 5
# baseline (speedup 1.0000x reference)
"""3-layer GCN on 8 Trainium2 NeuronCores (Bass/Tile).

Algorithm (mathematically equal to the reference up to fp reordering):
    dinv = 1/sqrt(deg)  with self-loops, deg computed on host from edge_index
    xs   = dinv * x     (row-scaled input, host)
    per layer:  z = A_raw @ act_s        (gather src rows + segment sum)
                zs = dinv_dst * z
                h  = zs @ W + b
                act_s = dinv * (elu(h) * mask/0.8)   [layers 1,2]
    out = zs3 @ W3 + b3

Distribution: nodes (and their incoming edges) are sharded across the 8
cores; every core keeps a full replica of the current activation table in
its HBM (all-gather per layer) and processes only its own dst rows.

The segment sum uses the indicator-matmul trick: edges sorted by dst are
processed in 128-edge chunks; a one-hot S^T[e, d] = (dst_local[e] == d)
matrix (built on VectorE with iota + is_equal) turns the scatter into a
TensorE matmul accumulated in PSUM.

Gathers use the SWDGE dma_gather instruction (int16 indices). Since
50000 > int16 max, edges are split per dst-block into two groups:
src < 32768 (gathered from table[0:]) and src >= 32768 (gathered from
table[32768:] with shifted indices).
"""

import numpy as np

N_NODES = 50000
N_EDGES = 800000
D = 512
C = 8                      # cores
SPLIT = 32768              # int16 index split point
SUB = 8                    # gather sub-batch size, in 128-edge chunks
P = 128

_compiled_cache = {}
TRACE = False          # set True (by test harness) to profile the HW run
LAST_RESULT = None     # BassKernelResults of the last run


# ----------------------------------------------------------------------------
# Host-side graph preprocessing
# ----------------------------------------------------------------------------

def _host_prep(edge_index, n, c):
    """Build the uniform per-core edge structure.

    Returns (structure, per_core_arrays, dinv) where structure describes the
    (identical across cores) chunk layout and per_core_arrays holds the int16
    gather indices / fp32 dst_local arrays per core.
    """
    r = n // c                       # rows per core
    nb = -(-r // P)                  # dst blocks per core
    src = np.concatenate([edge_index[0], np.arange(n, dtype=np.int64)])
    dst = np.concatenate([edge_index[1], np.arange(n, dtype=np.int64)])
    deg = np.bincount(dst, minlength=n)
    dinv = (1.0 / np.sqrt(deg.astype(np.float64))).astype(np.float32)

    # per (core, block, group) edge lists
    core_of = dst // r
    block_of = (dst % r) // P
    group_of = (src >= SPLIT).astype(np.int64)
    # sort edges by (core, block, group, src)
    order = np.lexsort((src, group_of, block_of, core_of))
    s_src = src[order]
    s_dst = dst[order]
    s_core = core_of[order]
    s_block = block_of[order]
    s_group = group_of[order]

    # counts[core, block, group]
    counts = np.zeros((c, nb, 2), np.int64)
    np.add.at(counts, (s_core, s_block, s_group), 1)
    # uniform chunk counts across cores
    nch = np.maximum(-(-counts.max(axis=0) // P), (counts.max(axis=0) > 0))  # [nb, 2]
    nch_tot = int(nch.sum())

    # slot each edge into its chunk position
    starts = np.zeros((c, nb, 2), np.int64)   # offsets of each (c,b,g) run
    flat = counts.reshape(-1)
    starts.reshape(-1)[1:] = np.cumsum(flat)[:-1]

    # chunk base (in units of chunks) for each (block, group), same per core
    chunk_base = np.zeros((nb, 2), np.int64)
    chunk_base.reshape(-1)[1:] = np.cumsum(nch.reshape(-1))[:-1]

    gidx = np.zeros((c, nch_tot, P), np.int16)        # pad idx = 0 (valid row)
    dstl = np.full((c, nch_tot, P), -1.0, np.float32)  # pad dst = -1 (no match)

    pos_in_run = np.arange(len(s_src)) - starts[s_core, s_block, s_group]
    chunk_id = chunk_base[s_block, s_group] + pos_in_run // P
    lane = pos_in_run % P
    idx_val = np.where(s_group == 0, s_src, s_src - SPLIT).astype(np.int16)
    gidx[s_core, chunk_id, lane] = idx_val
    dstl[s_core, chunk_id, lane] = (s_dst % r) % P

    # wrapped int16 index layout [128, 8*nch_tot] per core
    per_core = []
    for ci in range(c):
        w = gidx[ci].reshape(nch_tot, 8, 16)              # j = s*16 + p
        g16 = np.transpose(w, (2, 0, 1)).reshape(16, nch_tot * 8)
        gw = np.tile(g16, (8, 1)).astype(np.int16)        # [128, 8*nch_tot]
        dl = np.ascontiguousarray(dstl[ci].T)             # [128, nch_tot]
        per_core.append((gw, dl))

    structure = {
        "r": r, "nb": nb, "nch": nch, "chunk_base": chunk_base,
        "nch_tot": nch_tot,
    }
    return structure, per_core, dinv


# ----------------------------------------------------------------------------
# Bass program
# ----------------------------------------------------------------------------

def _build_program(structure, n, c, use_collectives=True):
    import concourse.bacc as bacc
    import concourse.tile as tile
    import concourse.mybir as mybir
    from concourse import library_config
    from concourse.masks import make_identity

    f32 = mybir.dt.float32
    i16 = mybir.dt.int16
    r, nb, nch, chunk_base, nch_tot = (
        structure["r"], structure["nb"], structure["nch"],
        structure["chunk_base"], structure["nch_tot"],
    )
    rpad = nb * P

    nc = bacc.Bacc("TRN2", target_bir_lowering=False, debug=False, num_devices=c)

    xs_t = nc.dram_tensor("xs", [n, D], f32, kind="ExternalInput")
    w_t = [nc.dram_tensor(f"W{i+1}", [D, D], f32, kind="ExternalInput") for i in range(3)]
    b_t = [nc.dram_tensor(f"b{i+1}", [D], f32, kind="ExternalInput") for i in range(3)]
    m_t = [nc.dram_tensor(f"M{i+1}", [rpad, D], f32, kind="ExternalInput") for i in range(2)]
    gidx_t = nc.dram_tensor("gidx", [P, 8 * nch_tot], i16, kind="ExternalInput")
    dstl_t = nc.dram_tensor("dstl", [P, nch_tot], f32, kind="ExternalInput")
    dinvb_t = nc.dram_tensor("dinvb", [P, nb], f32, kind="ExternalInput")
    iota_t = nc.dram_tensor("iota", [P, P], f32, kind="ExternalInput")
    out_t = nc.dram_tensor("out", [r, D], f32, kind="ExternalOutput")

    with tile.TileContext(nc) as tc:
        with tc.tile_pool(name="const", bufs=1) as cpool, \
             tc.tile_pool(name="msg", bufs=3) as msgp, \
             tc.tile_pool(name="sel", bufs=3) as selp, \
             tc.tile_pool(name="work", bufs=3) as workp, \
             tc.tile_pool(name="mask", bufs=2) as maskp, \
             tc.tile_pool(name="zps", bufs=2, space="PSUM") as zpsp, \
             tc.tile_pool(name="tps", bufs=2, space="PSUM") as tpsp, \
             tc.tile_pool(name="hps", bufs=2, space="PSUM") as hpsp, \
             tc.tile_pool(name="dram", bufs=1, space="DRAM") as dram:

            nc.gpsimd.load_library(library_config.attnmlp)

            # ---- constants -------------------------------------------------
            ident = cpool.tile([P, P], f32, tag="ident")
            make_identity(nc, ident[:])
            iota_sb = cpool.tile([P, P], f32, tag="iota")
            nc.sync.dma_start(out=iota_sb[:], in_=iota_t[:])
            gidx_sb = cpool.tile([P, 8 * nch_tot], i16, tag="gidx")
            nc.sync.dma_start(out=gidx_sb[:], in_=gidx_t[:])
            dstl_sb = cpool.tile([P, nch_tot], f32, tag="dstl")
            nc.sync.dma_start(out=dstl_sb[:], in_=dstl_t[:])
            dinvb_sb = cpool.tile([P, nb], f32, tag="dinvb")
            nc.sync.dma_start(out=dinvb_sb[:], in_=dinvb_t[:])
            ones_sb = cpool.tile([1, P], f32, tag="ones")
            nc.vector.memset(ones_sb[:], 1.0)
            w_sb = []
            for li in range(3):
                wl = cpool.tile([P, 4 * D], f32, tag=f"w{li}")
                for k in range(4):
                    nc.sync.dma_start(out=wl[:, k * D:(k + 1) * D],
                                      in_=w_t[li][k * P:(k + 1) * P, :])
                w_sb.append(wl)
            b_sb = []
            for li in range(3):
                bl = cpool.tile([1, D], f32, tag=f"b{li}")
                nc.sync.dma_start(out=bl[:], in_=b_t[li][None, :])
                b_sb.append(bl)

            # ---- DRAM buffers ---------------------------------------------
            if use_collectives:
                bounce = [dram.tile([r, D], f32, name=f"bounce{i}") for i in range(2)]
                acts = [dram.tile([n, D], f32, name=f"actbuf{i}", addr_space="Shared")
                        for i in range(2)]

            # ---- layers ----------------------------------------------------
            for li in range(3):
                if li == 0:
                    table = xs_t
                else:
                    table = acts[li - 1]

                for t in range(nb):
                    rows = min(P, r - t * P)
                    z_ps = zpsp.tile([P, D], f32, tag="zps")
                    # --- aggregate incoming edges ---
                    total_chunks = int(nch[t, 0] + nch[t, 1])
                    done = 0
                    for g in range(2):
                        cnt = int(nch[t, g])
                        if cnt == 0:
                            continue
                        c0 = int(chunk_base[t, g])
                        tbl = table[:, :] if g == 0 else table[SPLIT:, :]
                        for s0 in range(0, cnt, SUB):
                            nsb = min(SUB, cnt - s0)
                            cc0 = c0 + s0
                            msg = msgp.tile([P, SUB, D], f32, tag="msg")
                            nc.gpsimd.dma_gather(
                                msg[:, :nsb, :], tbl,
                                gidx_sb[:, 8 * cc0: 8 * (cc0 + nsb)],
                                P * nsb, P * nsb, D,
                            )
                            sel = selp.tile([P, SUB, P], f32, tag="sel")
                            for k in range(nsb):
                                nc.vector.tensor_tensor(
                                    out=sel[:, k, :],
                                    in0=dstl_sb[:, cc0 + k: cc0 + k + 1].to_broadcast([P, P]),
                                    in1=iota_sb[:],
                                    op=mybir.AluOpType.is_equal,
                                )
                            for k in range(nsb):
                                nc.tensor.matmul(
                                    z_ps[:], lhsT=sel[:, k, :], rhs=msg[:, k, :],
                                    start=(done == 0), stop=(done == total_chunks - 1),
                                )
                                done += 1
                    # --- zs = dinv_dst * z ---
                    zs = workp.tile([P, D], f32, tag="zs")
                    nc.scalar.activation(
                        zs[:], z_ps[:], mybir.ActivationFunctionType.Copy,
                        scale=dinvb_sb[:, t:t + 1],
                    )
                    # --- transpose zs -> zsT ---
                    t_ps = tpsp.tile([P, D], f32, tag="tps")
                    for k in range(4):
                        nc.tensor.transpose(
                            t_ps[:, k * P:(k + 1) * P], zs[:, k * P:(k + 1) * P], ident[:],
                        )
                    zst = workp.tile([P, D], f32, tag="zst")
                    nc.vector.tensor_copy(out=zst[:], in_=t_ps[:])
                    # --- h = zs @ W + b ---
                    h_ps = hpsp.tile([P, D], f32, tag="hps")
                    for k in range(4):
                        nc.tensor.matmul(
                            h_ps[:], lhsT=zst[:, k * P:(k + 1) * P],
                            rhs=w_sb[li][:, k * D:(k + 1) * D],
                            start=(k == 0), stop=False,
                        )
                    nc.tensor.matmul(
                        h_ps[:], lhsT=ones_sb[:1, :], rhs=b_sb[li][:1, :],
                        start=False, stop=True,
                    )
                    if li < 2:
                        # --- act = elu(h) * M  (M = mask*1.25*dinv) ---
                        ta = workp.tile([P, D], f32, tag="ta")
                        nc.vector.tensor_scalar_min(ta[:], h_ps[:], 0.0)
                        tb = workp.tile([P, D], f32, tag="tb")
                        nc.scalar.activation(tb[:], ta[:], mybir.ActivationFunctionType.Exp)
                        nc.vector.tensor_scalar_add(tb[:], tb[:], -1.0)
                        act = workp.tile([P, D], f32, tag="act")
                        nc.vector.tensor_tensor(out=act[:], in0=h_ps[:], in1=tb[:],
                                                op=mybir.AluOpType.max)
                        msk = maskp.tile([P, D], f32, tag="msk")
                        nc.sync.dma_start(out=msk[:], in_=m_t[li][t * P:(t + 1) * P, :])
                        nc.vector.tensor_tensor(out=act[:], in0=act[:], in1=msk[:],
                                                op=mybir.AluOpType.mult)
                        nc.sync.dma_start(out=bounce[li][t * P: t * P + rows, :],
                                          in_=act[:rows, :])
                    else:
                        res = workp.tile([P, D], f32, tag="res")
                        nc.vector.tensor_copy(out=res[:], in_=h_ps[:])
                        nc.sync.dma_start(out=out_t[t * P: t * P + rows, :],
                                          in_=res[:rows, :])
                if li < 2:
                    nc.gpsimd.collective_compute(
                        "AllGather", mybir.AluOpType.bypass,
                        replica_groups=[list(range(c))],
                        ins=[bounce[li][:]], outs=[acts[li][:]],
                    )

    nc.compile()
    return nc


# ----------------------------------------------------------------------------
# Entry point
# ----------------------------------------------------------------------------

def _dropout_masks(n, d):
    import jax
    cpu = jax.local_devices(backend="cpu")[0]
    with jax.default_device(cpu):
        dk = jax.random.key(42)
        k0 = np.asarray(jax.random.bernoulli(jax.random.fold_in(dk, 0), 0.8, (n, d)))
        k1 = np.asarray(jax.random.bernoulli(jax.random.fold_in(dk, 1), 0.8, (n, d)))
    return k0, k1


def _make_inputs(x, weights, biases, edge_index, structure, per_core, dinv, n, c):
    r, nb, nch_tot = structure["r"], structure["nb"], structure["nch_tot"]
    rpad = nb * P
    xs = (x * dinv[:, None]).astype(np.float32)
    k0, k1 = _dropout_masks(n, D)
    iota = np.broadcast_to(np.arange(P, dtype=np.float32), (P, P)).copy()

    in_maps = []
    for ci in range(c):
        gw, dl = per_core[ci]
        rows = slice(ci * r, (ci + 1) * r)
        dloc = dinv[rows]
        m1 = np.zeros((rpad, D), np.float32)
        m2 = np.zeros((rpad, D), np.float32)
        m1[:r] = k0[rows] * np.float32(1.25) * dloc[:, None]
        m2[:r] = k1[rows] * np.float32(1.25) * dloc[:, None]
        dinvb = np.zeros((P, nb), np.float32)
        dpad = np.zeros(rpad, np.float32)
        dpad[:r] = dloc
        dinvb[:, :] = dpad.reshape(nb, P).T
        in_maps.append({
            "xs": xs,
            "W1": weights[0], "W2": weights[1], "W3": weights[2],
            "b1": biases[0], "b2": biases[1], "b3": biases[2],
            "M1": m1, "M2": m2,
            "gidx": gw, "dstl": dl, "dinvb": dinvb, "iota": iota,
        })
    return in_maps


def kernel(x, W1, b1, W2, b2, W3, b3, edge_index):
    from concourse.bass_utils import run_bass_kernel_spmd

    x = np.asarray(x, dtype=np.float32)
    edge_index = np.asarray(edge_index)
    n = x.shape[0]

    structure, per_core, dinv = _host_prep(edge_index.astype(np.int64), n, C)

    key = ("gcn", n, structure["nch_tot"])
    if key not in _compiled_cache:
        _compiled_cache[key] = _build_program(structure, n, C)
    nc = _compiled_cache[key]

    in_maps = _make_inputs(
        x, [np.asarray(W1, np.float32), np.asarray(W2, np.float32), np.asarray(W3, np.float32)],
        [np.asarray(b1, np.float32), np.asarray(b2, np.float32), np.asarray(b3, np.float32)],
        edge_index, structure, per_core, dinv, n, C,
    )
    res = run_bass_kernel_spmd(nc, in_maps, core_ids=list(range(C)), trace=TRACE)
    global LAST_RESULT
    LAST_RESULT = res
    out = np.concatenate([res.results[ci]["out"] for ci in range(C)], axis=0)
    return out.astype(np.float32)


# revision 7
# speedup vs baseline: 1.6293x; 1.6293x over previous
"""3-layer GCN on 8 Trainium2 NeuronCores (Bass/Tile).

Algorithm (mathematically equal to the reference up to fp reordering):
    dinv = 1/sqrt(deg)  with self-loops, deg computed on host from edge_index
    xs   = dinv * x     (row-scaled input, host)
    per layer:  z = A_raw @ act_s        (gather src rows + segment sum)
                zs = dinv_dst * z
                h  = zs @ W + b
                act_s = dinv * (elu(h) * mask/0.8)   [layers 1,2]
    out = zs3 @ W3 + b3

Distribution: nodes (and their incoming edges) are sharded across the 8
cores; every core keeps a full replica of the current activation table in
its HBM (all-gather per layer) and processes only its own dst rows.

The segment sum uses the indicator-matmul trick: edges sorted by dst are
processed in 128-edge chunks; a one-hot S^T[e, d] = (dst_local[e] == d)
matrix (built on VectorE with iota + is_equal) turns the scatter into a
TensorE matmul accumulated in PSUM.

Gathers use the SWDGE dma_gather instruction (int16 indices). Since
50000 > int16 max, edges are split per dst-block into two groups:
src < 32768 (gathered from table[0:]) and src >= 32768 (gathered from
table[32768:] with shifted indices).
"""

import numpy as np

N_NODES = 50000
N_EDGES = 800000
D = 512
C = 8                      # cores
SPLIT = 32768              # int16 index split point
SUB = 8                    # gather sub-batch size, in 128-edge chunks
P = 128
BF16 = True                # bf16 storage + matmul compute (fp32 accumulate)
BATCH_ISEQ = True          # build S^T for a whole sub-batch in one DVE op

_compiled_cache = {}
TRACE = False          # set True (by test harness) to profile the HW run
LAST_RESULT = None     # BassKernelResults of the last run


# ----------------------------------------------------------------------------
# Host-side graph preprocessing
# ----------------------------------------------------------------------------

def _host_prep(edge_index, n, c):
    """Build the uniform per-core edge structure.

    Returns (structure, per_core_arrays, dinv) where structure describes the
    (identical across cores) chunk layout and per_core_arrays holds the int16
    gather indices / fp32 dst_local arrays per core.
    """
    r = n // c                       # rows per core
    nb = -(-r // P)                  # dst blocks per core
    src = np.concatenate([edge_index[0], np.arange(n, dtype=np.int64)])
    dst = np.concatenate([edge_index[1], np.arange(n, dtype=np.int64)])
    deg = np.bincount(dst, minlength=n)
    dinv = (1.0 / np.sqrt(deg.astype(np.float64))).astype(np.float32)

    # per (core, block, group) edge lists
    core_of = dst // r
    block_of = (dst % r) // P
    group_of = (src >= SPLIT).astype(np.int64)
    # sort edges by (core, block, group, src)
    order = np.lexsort((src, group_of, block_of, core_of))
    s_src = src[order]
    s_dst = dst[order]
    s_core = core_of[order]
    s_block = block_of[order]
    s_group = group_of[order]

    # counts[core, block, group]
    counts = np.zeros((c, nb, 2), np.int64)
    np.add.at(counts, (s_core, s_block, s_group), 1)
    # uniform chunk counts across cores
    nch = np.maximum(-(-counts.max(axis=0) // P), (counts.max(axis=0) > 0))  # [nb, 2]
    nch_tot = int(nch.sum())

    # slot each edge into its chunk position
    starts = np.zeros((c, nb, 2), np.int64)   # offsets of each (c,b,g) run
    flat = counts.reshape(-1)
    starts.reshape(-1)[1:] = np.cumsum(flat)[:-1]

    # chunk base (in units of chunks) for each (block, group), same per core
    chunk_base = np.zeros((nb, 2), np.int64)
    chunk_base.reshape(-1)[1:] = np.cumsum(nch.reshape(-1))[:-1]

    gidx = np.zeros((c, nch_tot, P), np.int16)        # pad idx = 0 (valid row)
    dstl = np.full((c, nch_tot, P), -1.0, np.float32)  # pad dst = -1 (no match)

    pos_in_run = np.arange(len(s_src)) - starts[s_core, s_block, s_group]
    chunk_id = chunk_base[s_block, s_group] + pos_in_run // P
    lane = pos_in_run % P
    idx_val = np.where(s_group == 0, s_src, s_src - SPLIT).astype(np.int16)
    gidx[s_core, chunk_id, lane] = idx_val
    dstl[s_core, chunk_id, lane] = (s_dst % r) % P

    # wrapped int16 index layout [128, 8*nch_tot] per core
    per_core = []
    for ci in range(c):
        w = gidx[ci].reshape(nch_tot, 8, 16)              # j = s*16 + p
        g16 = np.transpose(w, (2, 0, 1)).reshape(16, nch_tot * 8)
        gw = np.tile(g16, (8, 1)).astype(np.int16)        # [128, 8*nch_tot]
        dl = np.ascontiguousarray(dstl[ci].T)             # [128, nch_tot]
        per_core.append((gw, dl))

    structure = {
        "r": r, "nb": nb, "nch": nch, "chunk_base": chunk_base,
        "nch_tot": nch_tot,
    }
    return structure, per_core, dinv


# ----------------------------------------------------------------------------
# Bass program
# ----------------------------------------------------------------------------

def _build_program(structure, n, c, use_collectives=True):
    import concourse.bacc as bacc
    import concourse.tile as tile
    import concourse.mybir as mybir
    from concourse import library_config
    from concourse.masks import make_identity

    f32 = mybir.dt.float32
    i16 = mybir.dt.int16
    cdt = mybir.dt.bfloat16 if BF16 else f32
    r, nb, nch, chunk_base, nch_tot = (
        structure["r"], structure["nb"], structure["nch"],
        structure["chunk_base"], structure["nch_tot"],
    )
    rpad = nb * P

    nc = bacc.Bacc("TRN2", target_bir_lowering=False, debug=False, num_devices=c)

    xs_t = nc.dram_tensor("xs", [n, D], cdt, kind="ExternalInput")
    w_t = [nc.dram_tensor(f"W{i+1}", [D, D], cdt, kind="ExternalInput") for i in range(3)]
    b_t = [nc.dram_tensor(f"b{i+1}", [D], cdt, kind="ExternalInput") for i in range(3)]
    m_t = [nc.dram_tensor(f"M{i+1}", [rpad, D], f32, kind="ExternalInput") for i in range(2)]
    gidx_t = nc.dram_tensor("gidx", [P, 8 * nch_tot], i16, kind="ExternalInput")
    dstl_t = nc.dram_tensor("dstl", [P, nch_tot], f32, kind="ExternalInput")
    dinvb_t = nc.dram_tensor("dinvb", [P, nb], f32, kind="ExternalInput")
    iota_t = nc.dram_tensor("iota", [P, P], f32, kind="ExternalInput")
    out_t = nc.dram_tensor("out", [r, D], f32, kind="ExternalOutput")

    with tile.TileContext(nc) as tc:
        with tc.tile_pool(name="const", bufs=1) as cpool, \
             tc.tile_pool(name="msg", bufs=3) as msgp, \
             tc.tile_pool(name="sel", bufs=3) as selp, \
             tc.tile_pool(name="work", bufs=3) as workp, \
             tc.tile_pool(name="mask", bufs=2) as maskp, \
             tc.tile_pool(name="zps", bufs=2, space="PSUM") as zpsp, \
             tc.tile_pool(name="tps", bufs=2, space="PSUM") as tpsp, \
             tc.tile_pool(name="hps", bufs=2, space="PSUM") as hpsp, \
             tc.tile_pool(name="dram", bufs=1, space="DRAM") as dram:

            nc.gpsimd.load_library(library_config.attnmlp)

            # ---- constants -------------------------------------------------
            ident = cpool.tile([P, P], cdt, tag="ident")
            make_identity(nc, ident[:])
            iota_sb = cpool.tile([P, P], f32, tag="iota")
            nc.sync.dma_start(out=iota_sb[:], in_=iota_t[:])
            gidx_sb = cpool.tile([P, 8 * nch_tot], i16, tag="gidx")
            nc.sync.dma_start(out=gidx_sb[:], in_=gidx_t[:])
            dstl_sb = cpool.tile([P, nch_tot], f32, tag="dstl")
            nc.sync.dma_start(out=dstl_sb[:], in_=dstl_t[:])
            dinvb_sb = cpool.tile([P, nb], f32, tag="dinvb")
            nc.sync.dma_start(out=dinvb_sb[:], in_=dinvb_t[:])
            ones_sb = cpool.tile([1, P], cdt, tag="ones")
            nc.vector.memset(ones_sb[:], 1.0)
            w_sb = []
            for li in range(3):
                wl = cpool.tile([P, 4 * D], cdt, tag=f"w{li}")
                for k in range(4):
                    nc.sync.dma_start(out=wl[:, k * D:(k + 1) * D],
                                      in_=w_t[li][k * P:(k + 1) * P, :])
                w_sb.append(wl)
            b_sb = []
            for li in range(3):
                bl = cpool.tile([1, D], cdt, tag=f"b{li}")
                nc.sync.dma_start(out=bl[:], in_=b_t[li][None, :])
                b_sb.append(bl)

            # ---- DRAM buffers ---------------------------------------------
            if use_collectives:
                bounce = [dram.tile([r, D], cdt, name=f"bounce{i}") for i in range(2)]
                acts = [dram.tile([n, D], cdt, name=f"actbuf{i}", addr_space="Shared")
                        for i in range(2)]

            # ---- layers ----------------------------------------------------
            for li in range(3):
                if li == 0:
                    table = xs_t
                else:
                    table = acts[li - 1]

                for t in range(nb):
                    rows = min(P, r - t * P)
                    z_ps = zpsp.tile([P, D], f32, tag="zps")
                    # --- aggregate incoming edges ---
                    total_chunks = int(nch[t, 0] + nch[t, 1])
                    done = 0
                    for g in range(2):
                        cnt = int(nch[t, g])
                        if cnt == 0:
                            continue
                        c0 = int(chunk_base[t, g])
                        tbl = table[:, :] if g == 0 else table[SPLIT:, :]
                        for s0 in range(0, cnt, SUB):
                            nsb = min(SUB, cnt - s0)
                            cc0 = c0 + s0
                            msg = msgp.tile([P, SUB, D], cdt, tag="msg")
                            nc.gpsimd.dma_gather(
                                msg[:, :nsb, :], tbl,
                                gidx_sb[:, 8 * cc0: 8 * (cc0 + nsb)],
                                P * nsb, P * nsb, D,
                            )
                            sel = selp.tile([P, SUB, P], cdt, tag="sel")
                            if BATCH_ISEQ:
                                nc.vector.tensor_tensor(
                                    out=sel[:, :nsb, :],
                                    in0=dstl_sb[:, cc0:cc0 + nsb]
                                        .rearrange("p (c o) -> p c o", o=1)
                                        .to_broadcast([P, nsb, P]),
                                    in1=iota_sb[:]
                                        .rearrange("p (o d) -> p o d", o=1)
                                        .to_broadcast([P, nsb, P]),
                                    op=mybir.AluOpType.is_equal,
                                )
                            else:
                                for k in range(nsb):
                                    nc.vector.tensor_tensor(
                                        out=sel[:, k, :],
                                        in0=dstl_sb[:, cc0 + k: cc0 + k + 1].to_broadcast([P, P]),
                                        in1=iota_sb[:],
                                        op=mybir.AluOpType.is_equal,
                                    )
                            for k in range(nsb):
                                nc.tensor.matmul(
                                    z_ps[:], lhsT=sel[:, k, :], rhs=msg[:, k, :],
                                    start=(done == 0), stop=(done == total_chunks - 1),
                                )
                                done += 1
                    # --- zs = dinv_dst * z ---
                    zs = workp.tile([P, D], cdt, tag="zs")
                    nc.scalar.activation(
                        zs[:], z_ps[:], mybir.ActivationFunctionType.Copy,
                        scale=dinvb_sb[:, t:t + 1],
                    )
                    # --- transpose zs -> zsT ---
                    t_ps = tpsp.tile([P, D], cdt, tag="tps")
                    for k in range(4):
                        nc.tensor.transpose(
                            t_ps[:, k * P:(k + 1) * P], zs[:, k * P:(k + 1) * P], ident[:],
                        )
                    zst = workp.tile([P, D], cdt, tag="zst")
                    nc.vector.tensor_copy(out=zst[:], in_=t_ps[:])
                    # --- h = zs @ W + b ---
                    h_ps = hpsp.tile([P, D], f32, tag="hps")
                    for k in range(4):
                        nc.tensor.matmul(
                            h_ps[:], lhsT=zst[:, k * P:(k + 1) * P],
                            rhs=w_sb[li][:, k * D:(k + 1) * D],
                            start=(k == 0), stop=False,
                        )
                    nc.tensor.matmul(
                        h_ps[:], lhsT=ones_sb[:1, :], rhs=b_sb[li][:1, :],
                        start=False, stop=True,
                    )
                    if li < 2:
                        # --- act = elu(h) * M  (M = mask*1.25*dinv) ---
                        ta = workp.tile([P, D], f32, tag="ta")
                        nc.vector.tensor_scalar_min(ta[:], h_ps[:], 0.0)
                        tb = workp.tile([P, D], f32, tag="tb")
                        nc.scalar.activation(tb[:], ta[:], mybir.ActivationFunctionType.Exp)
                        nc.vector.tensor_scalar_add(tb[:], tb[:], -1.0)
                        act = workp.tile([P, D], f32, tag="actf")
                        nc.vector.tensor_tensor(out=act[:], in0=h_ps[:], in1=tb[:],
                                                op=mybir.AluOpType.max)
                        msk = maskp.tile([P, D], f32, tag="msk")
                        nc.sync.dma_start(out=msk[:], in_=m_t[li][t * P:(t + 1) * P, :])
                        actc = workp.tile([P, D], cdt, tag="actc")
                        nc.vector.tensor_tensor(out=actc[:], in0=act[:], in1=msk[:],
                                                op=mybir.AluOpType.mult)
                        nc.sync.dma_start(out=bounce[li][t * P: t * P + rows, :],
                                          in_=actc[:rows, :])
                    else:
                        res = workp.tile([P, D], f32, tag="res")
                        nc.vector.tensor_copy(out=res[:], in_=h_ps[:])
                        nc.sync.dma_start(out=out_t[t * P: t * P + rows, :],
                                          in_=res[:rows, :])
                if li < 2:
                    nc.gpsimd.collective_compute(
                        "AllGather", mybir.AluOpType.bypass,
                        replica_groups=[list(range(c))],
                        ins=[bounce[li][:]], outs=[acts[li][:]],
                    )

    nc.compile()
    return nc


# ----------------------------------------------------------------------------
# Entry point
# ----------------------------------------------------------------------------

def _dropout_masks(n, d):
    import jax
    cpu = jax.local_devices(backend="cpu")[0]
    with jax.default_device(cpu):
        dk = jax.random.key(42)
        k0 = np.asarray(jax.random.bernoulli(jax.random.fold_in(dk, 0), 0.8, (n, d)))
        k1 = np.asarray(jax.random.bernoulli(jax.random.fold_in(dk, 1), 0.8, (n, d)))
    return k0, k1


def _make_inputs(x, weights, biases, edge_index, structure, per_core, dinv, n, c):
    import ml_dtypes

    r, nb, nch_tot = structure["r"], structure["nb"], structure["nch_tot"]
    rpad = nb * P
    cnp = ml_dtypes.bfloat16 if BF16 else np.float32
    xs = (x * dinv[:, None]).astype(cnp)
    weights = [w.astype(cnp) for w in weights]
    biases = [b.astype(cnp) for b in biases]
    k0, k1 = _dropout_masks(n, D)
    iota = np.broadcast_to(np.arange(P, dtype=np.float32), (P, P)).copy()

    in_maps = []
    for ci in range(c):
        gw, dl = per_core[ci]
        rows = slice(ci * r, (ci + 1) * r)
        dloc = dinv[rows]
        m1 = np.zeros((rpad, D), np.float32)
        m2 = np.zeros((rpad, D), np.float32)
        m1[:r] = k0[rows] * np.float32(1.25) * dloc[:, None]
        m2[:r] = k1[rows] * np.float32(1.25) * dloc[:, None]
        dinvb = np.zeros((P, nb), np.float32)
        dpad = np.zeros(rpad, np.float32)
        dpad[:r] = dloc
        dinvb[:, :] = dpad.reshape(nb, P).T
        in_maps.append({
            "xs": xs,
            "W1": weights[0], "W2": weights[1], "W3": weights[2],
            "b1": biases[0], "b2": biases[1], "b3": biases[2],
            "M1": m1, "M2": m2,
            "gidx": gw, "dstl": dl, "dinvb": dinvb, "iota": iota,
        })
    return in_maps


def kernel(x, W1, b1, W2, b2, W3, b3, edge_index):
    from concourse.bass_utils import run_bass_kernel_spmd

    x = np.asarray(x, dtype=np.float32)
    edge_index = np.asarray(edge_index)
    n = x.shape[0]

    structure, per_core, dinv = _host_prep(edge_index.astype(np.int64), n, C)

    key = ("gcn", n, structure["nch_tot"])
    if key not in _compiled_cache:
        _compiled_cache[key] = _build_program(structure, n, C)
    nc = _compiled_cache[key]

    in_maps = _make_inputs(
        x, [np.asarray(W1, np.float32), np.asarray(W2, np.float32), np.asarray(W3, np.float32)],
        [np.asarray(b1, np.float32), np.asarray(b2, np.float32), np.asarray(b3, np.float32)],
        edge_index, structure, per_core, dinv, n, C,
    )
    res = run_bass_kernel_spmd(nc, in_maps, core_ids=list(range(C)), trace=TRACE)
    global LAST_RESULT
    LAST_RESULT = res
    out = np.concatenate([res.results[ci]["out"] for ci in range(C)], axis=0)
    return out.astype(np.float32)
